# revision 9
# baseline (speedup 1.0000x reference)
"""Trainium2 Bass kernel for nn_Cycle_Consistency_Loss (soft-DTW-style
cycle loss). Self-contained: host-side packing + SPMD Bass program on 8
NeuronCores + host reduction.

Math (per pair (a,b), both directions; x = seq[q], y = seq[k], lens = src_len//4):
  alpha = softmax_j(-|x_i-y_j|^2) over valid j -> snn = alpha @ y
  beta  = softmax_k(-|snn_i-x_k|^2) over valid k
  u = E_beta[k], std = E_beta[(k-u)^2]
  li = (i-u)^2/std + 0.005*ln(std), summed over valid i; total / n_pairs.

All matmuls run in bf16 (1 cycle/row on the PE vs 4 for fp32) with hi/lo
split operands: score = 2xh*yh + 2xh*yl + 2xl*yh - y2h - y2l over ~98
contraction rows, which costs the same as 34 rows (PE cost scales with
the output free dim only) but keeps ~fp32 accuracy since bf16 x bf16
products accumulate exactly in fp32 PSUM. The per-query -x2 term of
pass A cancels in the softmax and is dropped.

u comes from chunk-centered first moments (exact bf16 products,
per-chunk centers recombined by one fp32 matmul per step), so u is the
exact mean of the bf16 beta distribution. std uses the numerically
stable second sweep sum_k P2*(u-k)^2 with the subtraction done before
squaring; its three elementwise ops are split across DVE and GpSimd
with bf16 2x-mode where possible.
"""
import sys
import numpy as np

sys.path.insert(0, "/opt/trn_rl_repo")

QB = 512          # query block = matmul free dim = one PSUM bank of fp32
KG = 256          # key group (2 chunks of 128 partitions)
KR = 98           # contraction rows of the score matmuls
NCORES = 8
PENALTY = 0.01
BIG = 1.0e30
STD_FLOOR = 1.0e-35


def _ceil(a, b):
    return -(-a // b)


class _Item:
    __slots__ = ("qi", "ki", "Lq", "Lk", "qb", "ga", "gb", "dummy")

    def __init__(self, qi, ki, Lq, Lk, qb):
        self.qi, self.ki, self.Lq, self.Lk, self.qb = qi, ki, Lq, Lk, qb
        self.ga = _ceil(Lk, KG)
        self.gb = _ceil(Lq, KG)
        self.dummy = False


class _Dummy:
    qi = ki = Lq = Lk = qb = 0
    ga = gb = 0
    dummy = True


def _hilo(a):
    """Split fp32 array into bf16 hi + bf16 lo."""
    import ml_dtypes

    BF = ml_dtypes.bfloat16
    h = a.astype(BF)
    l = (a - h.astype(np.float32)).astype(BF)
    return h, l


def pack(seq, src_len, combinations):
    """Build the step plan and per-core input arrays.

    Per-core inputs:
      kA  [98, CA] bf16  pass-A key rows [yh; yl; yh; y2h; y2l] (mask y2h=BIG)
      vAr [128, CA//128*33] bf16  pass-A values, pre-swizzled:
                     vAr[p, g*33+d] = vA[g*128+p, d], vA = [y | 1]
      qA  [98, QB*NS] bf16 pass-A query rows [2xh; 2xh; 2xl; -1; -1]
      kB  [98, CB] bf16  pass-B key rows [2xh; 2xl; 2xh; x2h; x2l] (mask x2h=BIG)
      tw  [128, 64*32] bf16  chunk moment lhsT: block ch has [1; (p-63.5)]
                     in columns (2ch, 2ch+1), zero elsewhere, so the
                     accumulating matmul lands each chunk's [S0; S1c]
                     on PSUM rows (2ch, 2ch+1)
      cvec [64, 2] f32   per-chunk recombine lhsT: rows (2c,2c+1) =
                     [[128c+63.5, 1], [1, 0]]
      kcol [128, 32] f32 kcol[p, c] = 128c + p
      onesb [128, 1] bf16, onesf [128, 1] f32
      qidx/qmask [128, 4*NS] f32 absolute query index / valid mask
    """
    import ml_dtypes

    BF = ml_dtypes.bfloat16
    seq = np.asarray(seq, np.float32)
    lens = (np.asarray(src_len).astype(np.int64) // 4).astype(np.int64)
    comb = np.asarray(combinations).astype(np.int64)

    items = []
    for a, b in comb:
        for qi, ki in ((a, b), (b, a)):
            Lq, Lk = int(lens[qi]), int(lens[ki])
            if Lq <= 0 or Lk <= 0:
                continue
            for qb in range(_ceil(Lq, QB)):
                items.append(_Item(int(qi), int(ki), Lq, Lk, qb))
    items.sort(key=lambda it: -(it.ga + it.gb))
    NS = max(1, _ceil(len(items), NCORES))
    while len(items) < NS * NCORES:
        items.append(_Dummy())

    GA = [max(max(items[s * NCORES + c].ga for c in range(NCORES)), 1)
          for s in range(NS)]
    GB = [max(max(items[s * NCORES + c].gb for c in range(NCORES)), 1)
          for s in range(NS)]
    CA = sum(GA) * KG
    CB = sum(GB) * KG

    sq2 = np.einsum("btd,btd->bt", seq, seq).astype(np.float32)
    seqh, seql = _hilo(seq)      # [B, T, 32] bf16 each
    sq2h, sq2l = _hilo(sq2)      # [B, T]

    p = np.arange(128, dtype=np.float32)
    tw = np.zeros((128, 64 * 32), np.float32)
    for ch in range(32):
        tw[:, ch * 64 + 2 * ch] = 1.0
        tw[:, ch * 64 + 2 * ch + 1] = p - 63.5
    cvec = np.zeros((64, 2), np.float32)
    for c in range(32):
        cvec[2 * c, 0] = 128.0 * c + 63.5
        cvec[2 * c + 1, 0] = 1.0
        cvec[2 * c, 1] = 1.0
    kcol = np.zeros((128, 32), np.float32)
    for c in range(32):
        kcol[:, c] = 128.0 * c + p
    onesb = np.ones((128, 1), np.float32)
    onesf = np.ones((128, 1), np.float32)

    cores = []
    for c in range(NCORES):
        kA = np.zeros((KR, CA), BF)
        vA = np.zeros((CA, 33), np.float32)
        qA = np.zeros((KR, QB * NS), BF)
        kB = np.zeros((KR, CB), BF)
        qidx = np.zeros((128, 4 * NS), np.float32)
        qmask = np.zeros((128, 4 * NS), np.float32)
        offa = 0
        offb = 0
        its = []
        for s in range(NS):
            it = items[s * NCORES + c]
            its.append(it)
            na = GA[s] * KG
            nb = GB[s] * KG
            ka = kA[:, offa:offa + na]
            va = vA[offa:offa + na]
            kb = kB[:, offb:offb + nb]
            qa = qA[:, s * QB:(s + 1) * QB]
            if it.dummy:
                va[:, 32] = 1.0
            else:
                qi, ki = it.qi, it.ki
                Lk, Lq = it.Lk, it.Lq
                nk = min(Lk, na)
                ka[0:32, :nk] = seqh[ki, :nk].T
                ka[32:64, :nk] = seql[ki, :nk].T
                ka[64:96, :nk] = seqh[ki, :nk].T
                ka[96, :nk] = sq2h[ki, :nk]
                ka[97, :nk] = sq2l[ki, :nk]
                ka[96, nk:] = BIG
                va[:nk, 0:32] = seq[ki, :nk]
                va[:nk, 32] = 1.0
                q0 = it.qb * QB
                nq = min(Lq - q0, QB)
                xh2 = (2.0 * seqh[qi, q0:q0 + nq].astype(np.float32)).astype(BF)
                xl2 = (2.0 * seql[qi, q0:q0 + nq].astype(np.float32)).astype(BF)
                qa[0:32, :nq] = xh2.T
                qa[32:64, :nq] = xh2.T
                qa[64:96, :nq] = xl2.T
                qa[96, :nq] = -1.0
                qa[97, :nq] = -1.0
                nkb = min(Lq, nb)
                bxh2 = (2.0 * seqh[qi, :nkb].astype(np.float32)).astype(BF)
                bxl2 = (2.0 * seql[qi, :nkb].astype(np.float32)).astype(BF)
                kb[0:32, :nkb] = bxh2.T
                kb[32:64, :nkb] = bxl2.T
                kb[64:96, :nkb] = bxh2.T
                kb[96, :nkb] = sq2h[qi, :nkb]
                kb[97, :nkb] = sq2l[qi, :nkb]
                kb[96, nkb:] = BIG
                for c4 in range(4):
                    ii = q0 + c4 * 128 + np.arange(128)
                    qidx[:, s * 4 + c4] = ii.astype(np.float32)
                    qmask[:, s * 4 + c4] = (ii < Lq).astype(np.float32)
            offa += na
            offb += nb
        vAr = np.ascontiguousarray(
            vA.reshape(CA // 128, 128, 33).transpose(1, 0, 2).reshape(128, -1))
        cores.append(dict(kA=kA, vAr=vAr.astype(BF), qA=qA, kB=kB,
                          tw=tw.astype(BF), cvec=cvec, kcol=kcol,
                          onesb=onesb.astype(BF), onesf=onesf,
                          qidx=qidx, qmask=qmask, items=its))
    plan = dict(NS=NS, GA=GA, GB=GB, CA=CA, CB=CB)
    return plan, cores


IN_KEYS = ("kA", "vAr", "qA", "kB", "tw", "cvec", "kcol", "onesb", "onesf",
           "qidx", "qmask")


def build_program(plan):
    """Build the SPMD Bass program for the given step plan."""
    import concourse.bass as bass
    import concourse.bacc as bacc
    import concourse.mybir as mybir
    import concourse.tile as tile

    F32 = mybir.dt.float32
    BF16 = mybir.dt.bfloat16
    AFT = mybir.ActivationFunctionType
    NS, GA, GB = plan["NS"], plan["GA"], plan["GB"]
    CA, CB = plan["CA"], plan["CB"]
    GBmax = max(GB)
    GAmax = max(GA)

    nc = bacc.Bacc("TRN2", target_bir_lowering=False, debug=False,
                   num_devices=NCORES)
    kA_d = nc.dram_tensor("kA", [KR, CA], BF16, kind="ExternalInput")
    vAr_d = nc.dram_tensor("vAr", [128, (CA // 128) * 33], BF16,
                           kind="ExternalInput")
    qA_d = nc.dram_tensor("qA", [KR, QB * NS], BF16, kind="ExternalInput")
    kB_d = nc.dram_tensor("kB", [KR, CB], BF16, kind="ExternalInput")
    tw_d = nc.dram_tensor("tw", [128, 64 * 32], BF16, kind="ExternalInput")
    cvec_d = nc.dram_tensor("cvec", [64, 2], F32, kind="ExternalInput")
    kcol_d = nc.dram_tensor("kcol", [128, 32], F32, kind="ExternalInput")
    onesb_d = nc.dram_tensor("onesb", [128, 1], BF16, kind="ExternalInput")
    onesf_d = nc.dram_tensor("onesf", [128, 1], F32, kind="ExternalInput")
    qidx_d = nc.dram_tensor("qidx", [128, 4 * NS], F32, kind="ExternalInput")
    qmask_d = nc.dram_tensor("qmask", [128, 4 * NS], F32, kind="ExternalInput")
    out_d = nc.dram_tensor("out", [1, 1], F32, kind="ExternalOutput")

    with tile.TileContext(nc) as tc:
        with (
            tc.tile_pool(name="keys", bufs=2) as keys_pool,
            tc.tile_pool(name="vals", bufs=2) as vals_pool,
            tc.tile_pool(name="qrys", bufs=2) as qrys_pool,
            tc.tile_pool(name="pa", bufs=2) as pa_pool,
            tc.tile_pool(name="cache", bufs=2) as cache_pool,
            tc.tile_pool(name="epi", bufs=2) as epi_pool,
            tc.tile_pool(name="b2", bufs=2) as b2_pool,
            tc.tile_pool(name="fin", bufs=1) as fin_pool,
            tc.tile_pool(name="sc_ps", bufs=2, space="PSUM") as sc_psum,
            tc.tile_pool(name="na_ps", bufs=1, space="PSUM") as na_psum,
            tc.tile_pool(name="t2_ps", bufs=1, space="PSUM") as t2_psum,
            tc.tile_pool(name="u2_ps", bufs=1, space="PSUM") as u2_psum,
            tc.tile_pool(name="sd_ps", bufs=1, space="PSUM") as sd_psum,
        ):
            tw = fin_pool.tile([128, 64 * 32], BF16)
            nc.sync.dma_start(tw[:], tw_d[:])
            cvec = fin_pool.tile([64, 2], F32)
            nc.sync.dma_start(cvec[:], cvec_d[:])
            kcol = fin_pool.tile([128, 32], F32)
            nc.sync.dma_start(kcol[:], kcol_d[:])
            onesb = fin_pool.tile([128, 1], BF16)
            nc.sync.dma_start(onesb[:], onesb_d[:])
            onesf = fin_pool.tile([128, 1], F32)
            nc.sync.dma_start(onesf[:], onesf_d[:])
            qidx = fin_pool.tile([128, 4 * NS], F32)
            nc.sync.dma_start(qidx[:], qidx_d[:])
            qmask = fin_pool.tile([128, 4 * NS], F32)
            nc.sync.dma_start(qmask[:], qmask_d[:])
            stats_u = fin_pool.tile([128, 4 * NS], F32)
            stats_s = fin_pool.tile([128, 4 * NS], F32)

            offa = 0
            offb = 0
            b2_rr = 0  # round-robin for the B2 d-op engine split
            for s in range(NS):
                ga, gb = GA[s], GB[s]
                na, nb = ga * KG, gb * KG
                # ---- load this step's operands
                kA_t = keys_pool.tile([KR, GAmax * KG], BF16, tag="kA")
                nc.sync.dma_start(kA_t[:, :na], kA_d[:, offa:offa + na])
                vA_t = vals_pool.tile([128, GAmax * 2 * 33], BF16, tag="vA")
                nc.sync.dma_start(
                    vA_t[:, :ga * 66],
                    vAr_d[:, (offa // 128) * 33:((offa + na) // 128) * 33])
                qA_t = qrys_pool.tile([KR, QB], BF16, tag="qA")
                nc.sync.dma_start(qA_t[:], qA_d[:, s * QB:(s + 1) * QB])
                kB_t = keys_pool.tile([KR, GBmax * KG], BF16, tag="kB")
                nc.sync.dma_start(kB_t[:, :nb], kB_d[:, offb:offb + nb])

                # ---- pass A: numA[0:32] = snn.T * Z, numA[32] = Z
                numA = na_psum.tile([33, QB], F32)
                for g in range(ga):
                    sc = sc_psum.tile([128, 2 * QB], F32, tag="sc")
                    P = pa_pool.tile([128, 2 * QB], BF16, tag="pa")
                    for h in range(2):
                        ch = 2 * g + h
                        nc.tensor.matmul(
                            sc[:, h * QB:(h + 1) * QB],
                            kA_t[:, ch * 128:(ch + 1) * 128],
                            qA_t[:],
                            start=True, stop=True)
                    nc.scalar.activation(P[:], sc[:], AFT.Exp)
                    for h in range(2):
                        ch = 2 * g + h
                        nc.tensor.matmul(
                            numA[:],
                            vA_t[:, ch * 33:(ch + 1) * 33],
                            P[:, h * QB:(h + 1) * QB],
                            start=(g == 0 and h == 0),
                            stop=(g == ga - 1 and h == 1))

                # ---- epilogue A: R2 = [snnh; snnh; snnl; -1; -1] bf16
                nsb = epi_pool.tile([33, QB], F32, tag="nsb")
                nc.vector.tensor_copy(nsb[:], numA[:])
                zrow = epi_pool.tile([1, QB], F32, tag="zrow")
                nc.sync.dma_start(zrow[:], nsb[32:33, :])
                rz0 = epi_pool.tile([1, QB], F32, tag="rz0")
                nc.vector.reciprocal(rz0[:], zrow[:])
                rb = epi_pool.tile([32, QB], F32, tag="rb")
                nc.gpsimd.partition_broadcast(rb[:], rz0[:])
                snnf = epi_pool.tile([32, QB], F32, tag="snnf")
                nc.vector.tensor_mul(snnf[:], nsb[0:32, :], rb[:])
                R2 = epi_pool.tile([KR, QB], BF16, tag="R2")
                nc.gpsimd.memset(R2[96:98, :], -1.0)
                nc.vector.tensor_copy(R2[0:32, :], snnf[:])
                nc.gpsimd.tensor_copy(R2[32:64, :], snnf[:])
                snnhf = epi_pool.tile([32, QB], F32, tag="snnhf")
                nc.gpsimd.tensor_copy(snnhf[:], R2[0:32, :])
                nc.vector.tensor_sub(R2[64:96, :], snnf[:], snnhf[:])

                # ---- pass B1: P2 cached bf16; T2 rows = per-chunk [S0; S1c]
                cache = cache_pool.tile([128, GBmax * 2 * QB], BF16, tag="p2c")
                T2 = t2_psum.tile([64, QB], F32, tag="T2")
                for g in range(gb):
                    sc2 = sc_psum.tile([128, 2 * QB], F32, tag="sc")
                    for h in range(2):
                        ch = 2 * g + h
                        nc.tensor.matmul(
                            sc2[:, h * QB:(h + 1) * QB],
                            kB_t[:, ch * 128:(ch + 1) * 128],
                            R2[:],
                            start=True, stop=True)
                    nc.scalar.activation(
                        cache[:, g * 2 * QB:(g + 1) * 2 * QB], sc2[:], AFT.Exp)
                    for h in range(2):
                        ch = 2 * g + h
                        nc.tensor.matmul(
                            T2[:],
                            tw[:, ch * 64:(ch + 1) * 64],
                            cache[:, ch * QB:(ch + 1) * QB],
                            start=(ch == 0),
                            stop=(ch == 2 * gb - 1))

                # ---- mid: u = (sum_c c*S0_c + S1c_c) / Z2, broadcast
                T2sb = epi_pool.tile([64, QB], F32, tag="T2sb")
                nc.vector.tensor_copy(T2sb[0:4 * gb, :], T2[0:4 * gb, :])
                U2 = u2_psum.tile([2, QB], F32, tag="U2")
                nc.tensor.matmul(U2[:], cvec[0:4 * gb, :], T2sb[0:4 * gb, :],
                                 start=True, stop=True)
                U2sb = epi_pool.tile([2, QB], F32, tag="U2sb")
                nc.vector.tensor_copy(U2sb[:], U2[:])
                z2row = epi_pool.tile([1, QB], F32, tag="z2row")
                nc.sync.dma_start(z2row[:], U2sb[1:2, :])
                rz2 = epi_pool.tile([1, QB], F32, tag="rz2")
                nc.vector.reciprocal(rz2[:], z2row[:])
                u0 = epi_pool.tile([1, QB], F32, tag="u0")
                nc.vector.tensor_mul(u0[:], U2sb[0:1, :], rz2[:])
                ub = epi_pool.tile([128, QB], F32, tag="ub")
                nc.gpsimd.partition_broadcast(ub[:], u0[:])

                # ---- pass B2: stdsum = sum_k P2 * (u-k)^2 (stable)
                stdsum = sd_psum.tile([1, QB], F32, tag="sd")
                for ch in range(2 * gb):
                    d = b2_pool.tile([128, QB], BF16, tag="d")
                    eng = nc.vector if (b2_rr % 3 == 2) else nc.gpsimd
                    b2_rr += 1
                    eng.tensor_scalar_sub(d[:], ub[:], kcol[:, ch:ch + 1])
                    sq = b2_pool.tile([128, QB], BF16, tag="sq")
                    nc.vector.tensor_mul(sq[:], d[:], d[:])
                    w = b2_pool.tile([128, QB], BF16, tag="w")
                    nc.vector.tensor_mul(
                        w[:], sq[:], cache[:, ch * QB:(ch + 1) * QB])
                    nc.tensor.matmul(
                        stdsum[:], onesb[:], w[:],
                        start=(ch == 0), stop=(ch == 2 * gb - 1))

                # ---- epilogue B: write u, std into stats via transpose-DMA
                sstd = epi_pool.tile([1, QB], F32, tag="sstd")
                nc.vector.tensor_mul(sstd[:], stdsum[:], rz2[:])
                for c4 in range(4):
                    nc.sync.dma_start(
                        stats_u[:, s * 4 + c4:s * 4 + c4 + 1],
                        u0[0:1, c4 * 128:(c4 + 1) * 128])
                    nc.sync.dma_start(
                        stats_s[:, s * 4 + c4:s * 4 + c4 + 1],
                        sstd[0:1, c4 * 128:(c4 + 1) * 128])
                offa += na
                offb += nb

            # ---- final: li = (i-u)^2/std + 0.005*ln(std), masked sum
            W = 4 * NS
            stdc = fin_pool.tile([128, W], F32)
            nc.vector.tensor_scalar_max(stdc[:], stats_s[:], STD_FLOOR)
            rstd = fin_pool.tile([128, W], F32)
            nc.vector.reciprocal(rstd[:], stdc[:])
            delta = fin_pool.tile([128, W], F32)
            nc.vector.tensor_sub(delta[:], qidx[:], stats_u[:])
            d2 = fin_pool.tile([128, W], F32)
            nc.vector.tensor_mul(d2[:], delta[:], delta[:])
            t1 = fin_pool.tile([128, W], F32)
            nc.vector.tensor_mul(t1[:], d2[:], rstd[:])
            lg = fin_pool.tile([128, W], F32)
            nc.scalar.activation(lg[:], stdc[:], AFT.Ln)
            lgs = fin_pool.tile([128, W], F32)
            nc.vector.tensor_scalar_mul(lgs[:], lg[:], 0.5 * PENALTY)
            li = fin_pool.tile([128, W], F32)
            nc.vector.tensor_add(li[:], t1[:], lgs[:])
            lim = fin_pool.tile([128, W], F32)
            nc.vector.tensor_mul(lim[:], li[:], qmask[:])
            rowsum = fin_pool.tile([128, 1], F32)
            nc.vector.reduce_sum(rowsum[:], lim[:],
                                 axis=mybir.AxisListType.X)
            tot = u2_psum.tile([1, 1], F32, tag="U2")
            nc.tensor.matmul(tot[:], rowsum[:], onesf[:],
                             start=True, stop=True)
            osb = fin_pool.tile([1, 1], F32)
            nc.vector.tensor_copy(osb[:], tot[:])
            nc.sync.dma_start(out_d[:], osb[:])

    nc.compile()
    return nc


def kernel(seq, src_len, combinations):
    from concourse.bass_utils import run_bass_kernel_spmd

    plan, cores = pack(seq, src_len, combinations)
    nc = build_program(plan)
    in_maps = [{k: ci[k] for k in IN_KEYS} for ci in cores]
    res = run_bass_kernel_spmd(nc, in_maps, list(range(NCORES)))
    tot = np.float32(0.0)
    for c in range(NCORES):
        tot += np.float32(res.results[c]["out"][0, 0])
    n_pairs = np.asarray(combinations).shape[0]
    return np.float32(tot / np.float32(n_pairs))
